# revision 1
# baseline (speedup 1.0000x reference)
"""Trainium2 Bass kernel for nn_Decoder (LSTM decoder, B=131072, H=64, 12 steps).

Data-parallel across 8 NeuronCores (batch sharded, weights replicated).

Math folding (host side, exact algebra):
  x_t = rel_{t-1} @ W_emb.T + b_emb enters gates only through W_ih @ x_t, so
    W_eff = W_hh + (W_ih @ W_emb) @ W_pos
    b_eff = b_ih + b_hh + W_ih @ b_emb + (W_ih @ W_emb) @ b_pos
  and the hot recurrence is gates_t = W_eff @ h_{t-1} + b_eff (t >= 1), with
  step 0 using W_hh on h_init plus (W_ih @ W_emb) @ obs_rel and a bias without
  the b_pos term.

Device layout: hidden-major, two batch strips packed in the 128 partitions
(rows 0:64 = strip A hidden, 64:128 = strip B hidden) so every ACT/DVE op runs
full 128 lanes; gate matmuls use block-diagonal weights (K=128).

Positions for all 12 steps are produced per batch-group by 13 accumulating
matmuls into one [96, GC] psum tile (t in the partition dim):
  rel_t  = W_pos @ h_t + b_pos
  curr_t = obs + (t+1) b_pos + W_pos @ sum_{tau<=t} h_tau
The packed [96, COLS] result is DMA'd out contiguously and unpacked on host.
"""

import numpy as np

PRED = 12
H = 64
B = 131072
NCORES = 8
BC = B // NCORES          # 16384 batch per core
COLS = BC // 2            # 8192 columns (2 strips per column)
GC = 1024                 # columns per group
NG = COLS // GC           # 8 groups
NT = 32                   # packing tiles per core (512 batch each)
FT = COLS // NT           # 256 cols per packing tile

F32 = np.float32

_CACHE = {}


def _build_program():
    import concourse.mybir as mybir
    from concourse import bacc
    from concourse.tile import TileContext
    from contextlib import ExitStack

    f32 = mybir.dt.float32
    # float32r storage is identical to float32; the PE streams it at 1
    # cycle/row (vs 4 for plain fp32 matmul), costing ~1e-4 rel err here.
    f32r = mybir.dt.float32r
    AF = mybir.ActivationFunctionType

    nc = bacc.Bacc()

    h0p = nc.dram_tensor("h0p", [128, COLS], f32r, kind="ExternalInput")
    c0p = nc.dram_tensor("c0p", [128, COLS], f32, kind="ExternalInput")
    obsrel = nc.dram_tensor("obsrel", [4, COLS], f32r, kind="ExternalInput")
    obsbias = nc.dram_tensor("obsbias", [5, COLS], f32r, kind="ExternalInput")
    wg0 = nc.dram_tensor("wg0", [128, 512], f32r, kind="ExternalInput")
    wg = nc.dram_tensor("wg", [128, 512], f32r, kind="ExternalInput")
    wx = nc.dram_tensor("wx", [4, 512], f32r, kind="ExternalInput")
    b0 = nc.dram_tensor("b0", [128, 4], f32, kind="ExternalInput")
    bN = nc.dram_tensor("bN", [128, 4], f32, kind="ExternalInput")
    wpos = nc.dram_tensor("wpos", [128, PRED * 96], f32r, kind="ExternalInput")
    wposb = nc.dram_tensor("wposb", [5, 96], f32r, kind="ExternalInput")
    posout = nc.dram_tensor("posout", [96, COLS], f32, kind="ExternalOutput")

    with ExitStack() as ctx:
        tc = ctx.enter_context(TileContext(nc))
        const = ctx.enter_context(tc.tile_pool(name="const", bufs=1))
        hpool = ctx.enter_context(tc.tile_pool(name="hpool", bufs=26))
        cpool = ctx.enter_context(tc.tile_pool(name="cpool", bufs=2))
        stage = ctx.enter_context(tc.tile_pool(name="stage", bufs=2))
        obspool = ctx.enter_context(tc.tile_pool(name="obspool", bufs=2))
        ospool = ctx.enter_context(tc.tile_pool(name="ospool", bufs=2))
        gpsum = ctx.enter_context(tc.tile_pool(name="gpsum", bufs=3, space="PSUM"))
        ppsum = ctx.enter_context(tc.tile_pool(name="ppsum", bufs=1, space="PSUM"))

        # ---- resident weights ----
        wg0_s = const.tile([128, 512], f32r)
        wg_s = const.tile([128, 512], f32r)
        wx_s = const.tile([4, 512], f32r)
        b0_s = const.tile([128, 4], f32)
        bN_s = const.tile([128, 4], f32)
        wpos_s = const.tile([128, PRED * 96], f32r)
        wposb_s = const.tile([5, 96], f32r)
        nc.sync.dma_start(wg0_s[:], wg0[:, :])
        nc.sync.dma_start(wg_s[:], wg[:, :])
        nc.sync.dma_start(wx_s[:], wx[:, :])
        nc.sync.dma_start(b0_s[:], b0[:, :])
        nc.sync.dma_start(bN_s[:], bN[:, :])
        nc.sync.dma_start(wpos_s[:], wpos[:, :])
        nc.sync.dma_start(wposb_s[:], wposb[:, :])

        # phase order i, g, f, o; gate indices in weight layout: i=0 f=1 g=2 o=3
        PHASES = ((0, AF.Sigmoid, "si"), (2, AF.Tanh, "gg"),
                  (1, AF.Sigmoid, "sf"), (3, AF.Sigmoid, "so"))

        def emit_group_loads(g):
            sl = slice(g * GC, (g + 1) * GC)
            hs0 = hpool.tile([128, GC], f32r, tag="hs", name=f"hs_g{g}_t0")
            ct = cpool.tile([128, GC], f32, tag="c", name=f"c_g{g}")
            orl = obspool.tile([4, GC], f32r, tag="orl", name=f"orl_g{g}")
            obi = obspool.tile([5, GC], f32r, tag="obi", name=f"obi_g{g}")
            nc.sync.dma_start(hs0[:], h0p[:, sl])
            nc.sync.dma_start(ct[:], c0p[:, sl])
            nc.sync.dma_start(orl[:], obsrel[:, sl])
            nc.sync.dma_start(obi[:], obsbias[:, sl])
            return {"hs": [hs0], "c": ct, "orl": orl, "obi": obi}

        def emit_step(g, st, t):
            wsel = wg0_s if t == 0 else wg_s
            bsel = b0_s if t == 0 else bN_s
            acts = {}
            for gi, func, nm in PHASES:
                P = gpsum.tile([128, GC], f32, tag="gp", name=f"gp_{nm}_g{g}_t{t}")
                for h2 in range(2):
                    s2 = slice(512 * h2, 512 * (h2 + 1))
                    nc.tensor.matmul(
                        P[:, s2], lhsT=wsel[:, 128 * gi:128 * gi + 128],
                        rhs=st["hs"][t][:, s2], start=True, stop=(t != 0))
                    if t == 0:
                        nc.tensor.matmul(
                            P[:, s2], lhsT=wx_s[0:4, 128 * gi:128 * gi + 128],
                            rhs=st["orl"][0:4, s2], start=False, stop=True)
                A = stage.tile([128, GC], f32, tag=nm, name=f"{nm}_g{g}_t{t}")
                nc.scalar.activation(A[:], P[:], func, bias=bsel[:, gi:gi + 1])
                acts[nm] = A
                if nm == "gg":
                    # t1 = sigmoid(i) * tanh(g), in place over si
                    nc.vector.tensor_mul(acts["si"][:], acts["si"][:], A[:])
                elif nm == "sf":
                    c = st["c"]
                    nc.vector.tensor_mul(c[:], A[:], c[:])
                    nc.vector.tensor_add(c[:], c[:], acts["si"][:])
                    T = stage.tile([128, GC], f32, tag="tt", name=f"tt_g{g}_t{t}")
                    nc.scalar.activation(T[:], c[:], AF.Tanh)
                    acts["tt"] = T
            hn = hpool.tile([128, GC], f32r, tag="hs", name=f"hs_g{g}_t{t + 1}")
            nc.vector.tensor_mul(hn[:], acts["so"][:], acts["tt"][:])
            st["hs"].append(hn)

        def emit_pos(g, st):
            Pp = ppsum.tile([96, GC], f32, tag="pp", name=f"pp_g{g}")
            for h2 in range(2):
                s2 = slice(512 * h2, 512 * (h2 + 1))
                for t in range(PRED):
                    nc.tensor.matmul(
                        Pp[:, s2], lhsT=wpos_s[:, 96 * t:96 * t + 96],
                        rhs=st["hs"][t + 1][:, s2], start=(t == 0), stop=False)
                nc.tensor.matmul(
                    Pp[:, s2], lhsT=wposb_s[0:5, :], rhs=st["obi"][0:5, s2],
                    start=False, stop=True)
            S = ospool.tile([96, GC], f32, tag="os", name=f"os_g{g}")
            nc.vector.tensor_copy(S[:], Pp[:])
            nc.sync.dma_start(posout[:, g * GC:(g + 1) * GC], S[:])

        for pair in range(NG // 2):
            gA, gB = 2 * pair, 2 * pair + 1
            stA = emit_group_loads(gA)
            stB = emit_group_loads(gB)
            for t in range(PRED):
                emit_step(gA, stA, t)
                emit_step(gB, stB, t)
            emit_pos(gA, stA)
            emit_pos(gB, stB)

    nc.finalize()
    return nc


def _prep_inputs(encoder_h, encoder_c, obs_final_pos, obs_final_pos_rel,
                 W_emb, b_emb, W_ih, W_hh, b_ih, b_hh, W_pos, b_pos):
    f64 = np.float64
    W_emb, b_emb = W_emb.astype(f64), b_emb.astype(f64)
    W_ih, W_hh = W_ih.astype(f64), W_hh.astype(f64)
    b_ih, b_hh = b_ih.astype(f64), b_hh.astype(f64)
    W_pos, b_pos = W_pos.astype(f64), b_pos.astype(f64)

    W_ihe = W_ih @ W_emb                     # [256, 2]
    W_eff = W_hh + W_ihe @ W_pos             # [256, 64]
    b_eff0 = b_ih + b_hh + W_ih @ b_emb      # [256]
    b_effN = b_eff0 + W_ihe @ b_pos          # [256]

    def blockdiag_gates(W):
        # -> [128, 4*128]: per gate gi, cols 128*gi:+128 = blockdiag(Wg.T, Wg.T)
        out = np.zeros((128, 512), f64)
        for gi in range(4):
            Wg = W[64 * gi:64 * gi + 64, :]  # [64(out), 64(in)]
            out[0:64, 128 * gi:128 * gi + 64] = Wg.T
            out[64:128, 128 * gi + 64:128 * gi + 128] = Wg.T
        return out

    wg0 = blockdiag_gates(W_hh)
    wg = blockdiag_gates(W_eff)

    wx = np.zeros((4, 512), f64)
    for gi in range(4):
        Wg = W_ihe[64 * gi:64 * gi + 64, :]  # [64, 2]
        wx[0:2, 128 * gi:128 * gi + 64] = Wg.T
        wx[2:4, 128 * gi + 64:128 * gi + 128] = Wg.T

    b0 = np.zeros((128, 4), f64)
    bN = np.zeros((128, 4), f64)
    for gi in range(4):
        b0[:, gi] = np.tile(b_eff0[64 * gi:64 * gi + 64], 2)
        bN[:, gi] = np.tile(b_effN[64 * gi:64 * gi + 64], 2)

    # pos weights: psum rows m = half*48 + t'*4 + s*2 + k
    wpos = np.zeros((128, PRED * 96), f64)
    for t in range(PRED):
        Wt = np.zeros((128, 96), f64)
        for s in range(2):
            for k in range(2):
                for tp in range(PRED):
                    if tp == t:
                        Wt[64 * s:64 * s + 64, 0 * 48 + tp * 4 + s * 2 + k] = W_pos[k, :]
                    if tp >= t:
                        Wt[64 * s:64 * s + 64, 1 * 48 + tp * 4 + s * 2 + k] = W_pos[k, :]
        wpos[:, 96 * t:96 * t + 96] = Wt

    wposb = np.zeros((5, 96), f64)
    for tp in range(PRED):
        for s in range(2):
            for k in range(2):
                wposb[0, 0 * 48 + tp * 4 + s * 2 + k] = b_pos[k]
                wposb[0, 1 * 48 + tp * 4 + s * 2 + k] = (tp + 1) * b_pos[k]
                wposb[1 + 2 * s + k, 1 * 48 + tp * 4 + s * 2 + k] = 1.0

    h_all = np.asarray(encoder_h, F32)[0]     # [B, 64]
    c_all = np.asarray(encoder_c, F32)[0]
    obs = np.asarray(obs_final_pos, F32)      # [B, 2]
    obsr = np.asarray(obs_final_pos_rel, F32)

    def pack_state(X, rows):
        # per core: [BC, rows] -> [2*rows, COLS] with strip packing
        X = X.reshape(NCORES, NT, 2, FT, rows)
        return X.transpose(0, 2, 4, 1, 3).reshape(NCORES, 2 * rows, COLS)

    h0p = pack_state(h_all, H)
    c0p = pack_state(c_all, H)
    orl = pack_state(obsr, 2)
    obsp = pack_state(obs, 2)
    obi = np.concatenate(
        [np.ones((NCORES, 1, COLS), F32), obsp], axis=1)  # [NCORES, 5, COLS]

    consts = dict(
        wg0=np.ascontiguousarray(wg0, F32), wg=np.ascontiguousarray(wg, F32),
        wx=np.ascontiguousarray(wx, F32), b0=np.ascontiguousarray(b0, F32),
        bN=np.ascontiguousarray(bN, F32), wpos=np.ascontiguousarray(wpos, F32),
        wposb=np.ascontiguousarray(wposb, F32))

    in_maps = []
    for cid in range(NCORES):
        m = dict(consts)
        m["h0p"] = np.ascontiguousarray(h0p[cid])
        m["c0p"] = np.ascontiguousarray(c0p[cid])
        m["obsrel"] = np.ascontiguousarray(orl[cid])
        m["obsbias"] = np.ascontiguousarray(obi[cid])
        in_maps.append(m)
    return in_maps


def _unpack_outputs(results):
    rel_parts, cur_parts = [], []
    for cid in range(NCORES):
        po = results[cid]["posout"]  # [96, COLS]
        P = po.reshape(2, PRED, 2, 2, NT, FT)   # half, t, s, k, tile, j
        rel = P[0].transpose(0, 3, 1, 4, 2).reshape(PRED, BC, 2)
        cur = P[1].transpose(0, 3, 1, 4, 2).reshape(PRED, BC, 2)
        rel_parts.append(rel)
        cur_parts.append(cur)
    pred_rel = np.concatenate(rel_parts, axis=1)
    pred = np.concatenate(cur_parts, axis=1)
    return pred, pred_rel


def _run(in_maps, trace=False):
    from concourse import bass_utils
    if "nc" not in _CACHE:
        _CACHE["nc"] = _build_program()
    nc = _CACHE["nc"]
    res = bass_utils.run_bass_kernel_spmd(
        nc, in_maps, core_ids=list(range(NCORES)), trace=trace)
    return res


def kernel(**inputs):
    inputs = {k: np.asarray(v) for k, v in inputs.items()}
    in_maps = _prep_inputs(**inputs)
    res = _run(in_maps, trace=False)
    pred, pred_rel = _unpack_outputs(res.results)
    return pred.astype(F32), pred_rel.astype(F32)

